# revision 12
# baseline (speedup 1.0000x reference)
"""Multi-head attention kernel for Trainium2, SPMD over 8 NeuronCores.

Problem: qkv (8, 1536, 2048) f32 -> out (8, 512, 2048) f32
  B=8 batches, H=8 heads, C=64 channels/head, T=2048 tokens.
  out[b] = concat_h( softmax((q_h*s)^T (k_h*s)) applied to v_h )
  with s = C**-0.25 (i.e. scores scaled by C**-0.5 overall).

Sharding: batch b -> core b. Each core computes 8 heads independently;
no collectives needed.

Design (vs the f32->bf16 baseline):
  - QK runs in fp16 (1 col/cycle, 10-bit mantissa); the f32->fp16
    casts go on the ACT engine, which has slack -- the DVE (the
    baseline's cast engine) is needed for its exp share.
  - exp is split across two engines so the PE never waits on softmax:
    ACT does exact Exp (scale folded), DVE does Schraudolph exp -- a
    single tensor_scalar (p16 = bitcast_fp16(int16(s*A + B))) writing
    the fp16 bit pattern directly. Softmax normalization cancels the
    constant bias of the approximation; measured end-to-end rel err
    ~1.3e-2 < 2e-2.
  - AV in fp16 with stationary vt = [v^T | ones]: PSUM row 64 is
    the softmax denominator l; the DVE reciprocal reads it straight
    from PSUM (no l evacuation pass).
  - PSUM: 4 rotating score-quarter banks + 4 AV-quarter banks; AV
    quarters are separate tiles so the next head's AV only waits on
    the evacuation of its own quarter.
"""

import os
import sys

import numpy as np

for _p in ("/opt/trn_rl_repo", "/root/.axon_site/_ro/trn_rl_repo"):
    if os.path.isdir(_p) and _p not in sys.path:
        sys.path.insert(0, _p)

B, H, C, T = 8, 8, 64, 2048
HC = H * C  # 512
W = 3 * HC  # 1536
NCH = T // 128  # 16 key chunks of 128
NQ = 4  # t-quarters of 512
THALF = T // 2

# Schraudolph exp writing the 16-bit float's bit pattern via an int16
# convert: bits = round(0.125*s*M*log2(e) + bias + c), M = mantissa
# size, bias = bits of 1.0. +0.5 turns the (sim) truncation into
# rounding; c centers the sawtooth (softmax normalization cancels the
# common-mode part, so its value barely matters).
P16 = os.environ.get("P16", "bf16")  # bf16 | fp16 (fp16 untested on hw)
if P16 == "fp16":
    SCH_A = 0.125 * 1024 * 1.4426950408889634
    SCH_B = 15360.0 - 44.0 + 0.5
else:
    SCH_A = 0.125 * 128 * 1.4426950408889634
    SCH_B = 16256.0 - 5.5 + 0.5

# engine assignment per t-quarter for the exp pass
EXP_ENGINES = tuple(os.environ.get("EXP_ENGINES", "act,dve,act,dve").split(","))
# engine per t-quarter for PSUM->SBUF evacuation of av (gpsimd cannot
# read PSUM, so only act/dve are valid here)
EVAC_ENGINES = ("act", "act", "act", "act")
# engine for the final out = av * (1/l) multiply (per t-half)
MULT_ENGINE = "pool"

_CACHE = {}


def _build_nc():
    from contextlib import ExitStack

    import concourse.bass as bass
    import concourse.mybir as mybir
    from concourse import bacc
    from concourse.masks import make_identity
    from concourse.tile import TileContext

    f32 = mybir.dt.float32
    fp16 = mybir.dt.float16 if P16 == "fp16" else mybir.dt.bfloat16
    i16 = mybir.dt.int16
    Exp = mybir.ActivationFunctionType.Exp
    mult = mybir.AluOpType.mult
    add = mybir.AluOpType.add

    nc = bacc.Bacc("TRN2", target_bir_lowering=False, debug=False)
    qkv = nc.declare_dram_parameter("qkv", [W, T], f32, isOutput=False)
    out = nc.declare_dram_parameter("out", [HC, T], f32, isOutput=True)

    with TileContext(nc) as tc, ExitStack() as ctx:
        singles = ctx.enter_context(tc.tile_pool(name="singles", bufs=1))
        qkv_pool = ctx.enter_context(tc.tile_pool(name="qkvp", bufs=2))
        vt_pool = ctx.enter_context(tc.tile_pool(name="vtp", bufs=2))
        pt_pool = ctx.enter_context(tc.tile_pool(name="ptp", bufs=12))
        out_pool = ctx.enter_context(tc.tile_pool(name="outp", bufs=2))
        l_pool = ctx.enter_context(tc.tile_pool(name="lp", bufs=2))
        ps_sc = ctx.enter_context(tc.tile_pool(name="ps_sc", bufs=4, space="PSUM"))
        ps_av = ctx.enter_context(tc.tile_pool(name="ps_av", bufs=4, space="PSUM"))

        ident = singles.tile([128, 128], f32)
        make_identity(nc, ident)

        for pair in range(4):
            q2 = qkv_pool.tile([128, T], f32, tag="q2")
            k2 = qkv_pool.tile([128, T], f32, tag="k2")
            v2 = qkv_pool.tile([128, T], f32, tag="v2")
            q2h = qkv_pool.tile([128, T], fp16, tag="q2h")
            k2h = qkv_pool.tile([128, T], fp16, tag="k2h")
            r0 = pair * 128
            if pair == 0:
                # stage what QK_0 (head 0) needs first so the PE starts
                # as early as possible
                nc.sync.dma_start(out=k2[0:64, 0:128], in_=qkv[HC : HC + 64, 0:128])
                nc.sync.dma_start(out=q2[0:64, 0:THALF], in_=qkv[0:64, 0:THALF])
                nc.scalar.copy(k2h[0:64, 0:128], k2[0:64, 0:128])
                nc.scalar.copy(q2h[0:64, 0:THALF], q2[0:64, 0:THALF])
                nc.sync.dma_start(out=k2[0:64, 128:T], in_=qkv[HC : HC + 64, 128:T])
                nc.sync.dma_start(out=q2[0:64, THALF:T], in_=qkv[0:64, THALF:T])
                nc.scalar.copy(k2h[0:64, 128:T], k2[0:64, 128:T])
                nc.scalar.copy(q2h[0:64, THALF:T], q2[0:64, THALF:T])
                nc.sync.dma_start(out=v2, in_=qkv[2 * HC : 2 * HC + 128, :])
                nc.sync.dma_start(out=k2[64:128, :], in_=qkv[HC + 64 : HC + 128, :])
                nc.sync.dma_start(out=q2[64:128, :], in_=qkv[64:128, :])
                nc.scalar.copy(k2h[64:128, :], k2[64:128, :])
                nc.scalar.copy(q2h[64:128, :], q2[64:128, :])
            else:
                nc.sync.dma_start(out=q2, in_=qkv[r0 : r0 + 128, :])
                nc.sync.dma_start(out=k2, in_=qkv[HC + r0 : HC + r0 + 128, :])
                nc.sync.dma_start(
                    out=v2, in_=qkv[2 * HC + r0 : 2 * HC + r0 + 128, :]
                )
                nc.scalar.copy(k2h, k2)
                nc.scalar.copy(q2h, q2)

            # vt[s-part, chunk, 0:64] = v^T, [.., 64] = 1 (denominator row)
            vts = [
                vt_pool.tile([128, NCH, 65], fp16, tag=f"vt{i}", name=f"vt{i}")
                for i in range(2)
            ]
            for vt in vts:
                nc.vector.memset(vt[:, :, 64:65], 1.0)

            for hh in range(2):
                h = pair * 2 + hh
                o = hh * 64
                q = q2h[o : o + 64, :]
                k = k2h[o : o + 64, :]
                vt = vts[hh]

                avq = [
                    ps_av.tile([128, 512], f32, tag="av", name=f"av{i}")
                    for i in range(NQ)
                ]
                prev_pts = None
                for j in range(NCH):
                    kj = k[:, j * 128 : (j + 1) * 128]
                    pts = []
                    scs = []
                    for qq in range(NQ):
                        sc = ps_sc.tile([128, 512], f32, tag="sc")
                        scs.append(sc)
                        nc.tensor.matmul(
                            sc,
                            kj,
                            q[:, qq * 512 : (qq + 1) * 512],
                            start=True,
                            stop=True,
                        )
                    # v transposes ride in the PE slack of early chunks
                    # (first head of the pair only); [128,4,128] f32 fits
                    # the same 2KB PSUM slot as a score quarter
                    if hh == 0 and j < 4:
                        tb = ps_sc.tile([128, 4, 128], f32, tag="sc")
                        for jj in range(4):
                            cj = j * 4 + jj
                            nc.tensor.transpose(
                                tb[:, jj, :],
                                v2[:, cj * 128 : (cj + 1) * 128],
                                ident,
                            )
                        nc.vector.tensor_copy(
                            vts[0][:, j * 4 : (j + 1) * 4, 0:64], tb[:, :, 0:64]
                        )
                        nc.vector.tensor_copy(
                            vts[1][:, j * 4 : (j + 1) * 4, 0:64], tb[:, :, 64:128]
                        )
                    for qq in range(NQ):
                        pt = pt_pool.tile([128, 512], fp16)
                        pts.append(pt)
                        if EXP_ENGINES[qq] == "act":
                            nc.scalar.activation(pt, scs[qq], Exp, scale=0.125)
                        else:
                            nc.vector.tensor_scalar(
                                pt.bitcast(i16), scs[qq], SCH_A, SCH_B, mult, add
                            )
                    if prev_pts is not None:
                        for qq in range(NQ):
                            nc.tensor.matmul(
                                avq[qq][0:65, :],
                                vt[:, j - 1, :],
                                prev_pts[qq],
                                start=(j - 1 == 0),
                                stop=(j - 1 == NCH - 1),
                                skip_group_check=True,
                            )
                    prev_pts = pts
                for qq in range(NQ):
                    nc.tensor.matmul(
                        avq[qq][0:65, :],
                        vt[:, NCH - 1, :],
                        prev_pts[qq],
                        start=False,
                        stop=True,
                        skip_group_check=True,
                    )

                # evacuate + normalize: rows 0..63 = unnormalized out,
                # row 64 = l (read by the reciprocal straight from PSUM)
                av_sb = out_pool.tile([64, T], f32, tag="avsb")
                l_sb = l_pool.tile([1, T], f32, tag="lsb")
                rl = l_pool.tile([1, T], f32, tag="rl")
                for qq in range(NQ):
                    dst = av_sb[:, qq * 512 : (qq + 1) * 512]
                    srcq = avq[qq][0:64, :]
                    if EVAC_ENGINES[qq] == "act":
                        nc.scalar.copy(dst, srcq)
                    else:
                        nc.vector.tensor_copy(dst, srcq)
                    nc.vector.tensor_copy(
                        l_sb[:, qq * 512 : (qq + 1) * 512], avq[qq][64:65, :]
                    )
                    nc.vector.reciprocal_approx_fast(
                        out=rl[:, qq * 512 : (qq + 1) * 512],
                        in_=l_sb[:, qq * 512 : (qq + 1) * 512],
                    )
                rlb = l_pool.tile([64, T], f32, tag="rlb")
                o_sb = out_pool.tile([64, T], f32, tag="osb")
                for half in range(2):
                    t0, t1 = half * THALF, (half + 1) * THALF
                    nc.gpsimd.partition_broadcast(rlb[:, t0:t1], rl[:, t0:t1])
                    if MULT_ENGINE == "pool":
                        nc.gpsimd.tensor_mul(
                            o_sb[:, t0:t1], av_sb[:, t0:t1], rlb[:, t0:t1]
                        )
                    else:
                        nc.vector.tensor_mul(
                            o_sb[:, t0:t1], av_sb[:, t0:t1], rlb[:, t0:t1]
                        )
                    nc.sync.dma_start(
                        out=out[h * 64 : (h + 1) * 64, t0:t1], in_=o_sb[:, t0:t1]
                    )

    nc.finalize()
    return nc


def _get_nc():
    if "nc" not in _CACHE:
        _CACHE["nc"] = _build_nc()
    return _CACHE["nc"]


def _run(qkv_full, trace=False, tmpdir=None):
    """qkv_full: (8, 1536, 2048) f32. Returns (out (8,512,2048) f32, exec_ns)."""
    from concourse.bass_utils import run_bass_kernel_spmd

    nc = _get_nc()
    qkv_full = np.ascontiguousarray(np.asarray(qkv_full, dtype=np.float32))
    in_maps = [{"qkv": qkv_full[i]} for i in range(B)]
    res = run_bass_kernel_spmd(
        nc, in_maps, core_ids=list(range(B)), trace=trace, tmpdir=tmpdir
    )
    outs = np.stack([np.asarray(res.results[i]["out"]) for i in range(B)], axis=0)
    return outs, res.exec_time_ns


def kernel(qkv, n_heads=8):
    out, _ = _run(qkv)
    return out.astype(np.float32)


# revision 16
# speedup vs baseline: 1.4070x; 1.4070x over previous
"""Multi-head attention kernel for Trainium2, SPMD over 8 NeuronCores.

Problem: qkv (8, 1536, 2048) f32 -> out (8, 512, 2048) f32
  B=8 batches, H=8 heads, C=64 channels/head, T=2048 tokens.
  out[b] = concat_h( softmax((q_h*s)^T (k_h*s)) applied to v_h )
  with s = C**-0.25 (i.e. scores scaled by C**-0.5 overall).

Sharding: batch b -> core b. Each core computes 8 heads independently;
no collectives needed.

Design notes (what each engine does and why):
  - PE runs ONLY the QK and AV matmuls, two 1024-wide matmuls per
    chunk each, back to back. Fewer, wider matmuls amortize the
    serial LDWEIGHTS and per-instruction overhead that dominated
    narrower layouts, and an uninterrupted PE stream is required for
    the p-state ramp to reach the 2.4 GHz peak (any recurring stall
    parks the PE at 1.2 GHz or lower).
  - exp splits across ACT (exact Exp) and DVE (Schraudolph exp: one
    tensor_scalar writing the bf16 bit pattern through an int16
    convert). Softmax normalization cancels the approximation's
    common-mode bias; end-to-end rel err ~1.3e-2 (sim) < 2e-2.
  - v^T comes from DMA-transpose (XBAR) off a bf16 copy of v, not
    from PE transposes.
  - AV stationary vt = [v^T | ones]: PSUM row 64 of each av half is
    the softmax denominator l.
  - All input DMAs are issued up front on the SP queue (pure load
    stream, so prefetch never waits behind compute); output DMAs are
    issued from gpsimd right after its normalize multiply.
  - Tail work (evac, l copy, recip, broadcast, normalize) and the
    next pair's prep (casts, transpose DMAs) are emitted as
    background tasks interleaved into the NEXT head's chunk loop, so
    no engine's queue ever has a long-wait instruction parked at its
    head (head-of-line blocking stalled earlier revisions).
  - PSUM: 2x2 banks of rotating score halves + 2x2 banks of av
    halves.
"""

import os
import sys
from collections import deque

import numpy as np

for _p in ("/opt/trn_rl_repo", "/root/.axon_site/_ro/trn_rl_repo"):
    if os.path.isdir(_p) and _p not in sys.path:
        sys.path.insert(0, _p)

B, H, C, T = 8, 8, 64, 2048
HC = H * C  # 512
W = 3 * HC  # 1536
NCH = T // 128  # 16 key chunks of 128
TH = 1024  # half of T

# Schraudolph exp writing bf16 bits via an int16 convert:
# bits = round(0.125*s*128*log2(e) + 16256 + c). +0.5 makes a
# truncating convert round; c centers the sawtooth (mostly cancelled
# by softmax normalization anyway).
SCH_A = 0.125 * 128 * 1.4426950408889634
SCH_B = 16256.0 - 5.5 + 0.5

# engine per t-half for the exp pass
EXP_ENGINES = tuple(os.environ.get("EXP_ENGINES", "act,dve").split(","))
# moving-dim width per matmul instruction; 512 is the hardware limit
# (1024 fails ISA validation: s3d3_mm_num_elements)
MM_MOV = int(os.environ.get("MM_MOV", "512"))

_CACHE = {}


def _build_nc():
    from contextlib import ExitStack

    import concourse.mybir as mybir
    from concourse import bacc
    from concourse.tile import TileContext

    f32 = mybir.dt.float32
    bf16 = mybir.dt.bfloat16
    i16 = mybir.dt.int16
    Exp = mybir.ActivationFunctionType.Exp
    mul_op = mybir.AluOpType.mult
    add_op = mybir.AluOpType.add

    nc = bacc.Bacc("TRN2", target_bir_lowering=False, debug=False)
    qkv = nc.declare_dram_parameter("qkv", [W, T], f32, isOutput=False)
    out = nc.declare_dram_parameter("out", [HC, T], f32, isOutput=True)

    with TileContext(nc) as tc, ExitStack() as ctx:
        qkv_pool = ctx.enter_context(tc.tile_pool(name="qkvp", bufs=2))
        vt_pool = ctx.enter_context(tc.tile_pool(name="vtp", bufs=2))
        pt_pool = ctx.enter_context(tc.tile_pool(name="ptp", bufs=6))
        out_pool = ctx.enter_context(tc.tile_pool(name="outp", bufs=2))
        l_pool = ctx.enter_context(tc.tile_pool(name="lp", bufs=2))
        ps_sc = ctx.enter_context(tc.tile_pool(name="ps_sc", bufs=2, space="PSUM"))
        ps_av = ctx.enter_context(tc.tile_pool(name="ps_av", bufs=2, space="PSUM"))

        # ---- tiles for all 4 head-pairs (bufs=2 rotates the buffers;
        # the semaphores the DMAs inherit implement the prefetch gating)
        pairs = []
        for p in range(4):
            pr = {}
            pr["q2"] = qkv_pool.tile([128, T], f32, tag="q2", name="q2")
            pr["k2"] = qkv_pool.tile([128, T], f32, tag="k2", name="k2")
            pr["v2"] = qkv_pool.tile([128, T], f32, tag="v2", name="v2")
            pr["q2h"] = qkv_pool.tile([128, T], bf16, tag="q2h", name="q2h")
            pr["k2h"] = qkv_pool.tile([128, T], bf16, tag="k2h", name="k2h")
            pr["v2h"] = qkv_pool.tile([128, T], bf16, tag="v2h", name="v2h")
            pr["vt"] = [
                vt_pool.tile([128, NCH, 65], bf16, tag=f"vt{i}", name=f"vt{i}")
                for i in range(2)
            ]
            pr["vtt"] = [
                vt_pool.tile([128, NCH, 64], bf16, tag=f"vtt{i}", name=f"vtt{i}")
                for i in range(2)
            ]
            pairs.append(pr)

        # ---- all input DMAs up front, in consumption order
        for p, pr in enumerate(pairs):
            r0 = p * 128
            if p == 0:
                # first pair staged so QK_0 starts as early as possible
                nc.sync.dma_start(out=pr["k2"][0:64, 0:128], in_=qkv[HC : HC + 64, 0:128])
                nc.sync.dma_start(out=pr["q2"][0:64, 0:TH], in_=qkv[0:64, 0:TH])
                nc.sync.dma_start(out=pr["k2"][0:64, 128:T], in_=qkv[HC : HC + 64, 128:T])
                nc.sync.dma_start(out=pr["q2"][0:64, TH:T], in_=qkv[0:64, TH:T])
                nc.sync.dma_start(out=pr["v2"], in_=qkv[2 * HC : 2 * HC + 128, :])
                nc.sync.dma_start(out=pr["k2"][64:128, :], in_=qkv[HC + 64 : HC + 128, :])
                nc.sync.dma_start(out=pr["q2"][64:128, :], in_=qkv[64:128, :])
            else:
                nc.sync.dma_start(out=pr["k2"], in_=qkv[HC + r0 : HC + r0 + 128, :])
                nc.sync.dma_start(out=pr["q2"], in_=qkv[r0 : r0 + 128, :])
                nc.sync.dma_start(out=pr["v2"], in_=qkv[2 * HC + r0 : 2 * HC + r0 + 128, :])

        def emit_prep_tasks(p, staged):
            """Casts + v transpose + ones for pair p as a task list."""
            pr = pairs[p]
            k2, q2, v2 = pr["k2"], pr["q2"], pr["v2"]
            k2h, q2h, v2h = pr["k2h"], pr["q2h"], pr["v2h"]
            vt0, vt1 = pr["vt"]
            vtt0, vtt1 = pr["vtt"]
            tasks = []
            if staged:
                tasks += [
                    lambda: nc.scalar.copy(k2h[0:64, 0:128], k2[0:64, 0:128]),
                    lambda: nc.scalar.copy(q2h[0:64, 0:TH], q2[0:64, 0:TH]),
                    lambda: nc.scalar.copy(k2h[0:64, 128:T], k2[0:64, 128:T]),
                    lambda: nc.scalar.copy(q2h[0:64, TH:T], q2[0:64, TH:T]),
                ]
            else:
                tasks += [
                    lambda: nc.scalar.copy(k2h[0:64, :], k2[0:64, :]),
                    lambda: nc.scalar.copy(q2h[0:64, :], q2[0:64, :]),
                ]
            tasks += [
                lambda: nc.scalar.copy(k2h[64:128, :], k2[64:128, :]),
                lambda: nc.scalar.copy(q2h[64:128, :], q2[64:128, :]),
                lambda: nc.scalar.copy(v2h[0:64, :], v2[0:64, :]),
                lambda: nc.scalar.copy(v2h[64:128, :], v2[64:128, :]),
                lambda: nc.vector.memset(vt0[:, :, 64:65], 1.0),
                lambda: nc.vector.memset(vt1[:, :, 64:65], 1.0),
                # XBAR transpose to a contiguous tile (strided XBAR
                # output produced garbage on hw), then a plain DMA into
                # the 65-pitch vt layout
                lambda: nc.scalar.dma_start_transpose(vtt0, v2h[0:64, :]),
                lambda: nc.scalar.dma_start_transpose(vtt1, v2h[64:128, :]),
                lambda: nc.scalar.dma_start(out=vt0[:, :, 0:64], in_=vtt0),
                lambda: nc.scalar.dma_start(out=vt1[:, :, 0:64], in_=vtt1),
            ]
            return tasks

        def run_all(tasks):
            for t in tasks:
                t()

        # pair 0 prep inline at startup
        run_all(emit_prep_tasks(0, staged=True))

        pending_av = deque()  # closures for not-yet-emitted AV chunk pairs
        bg = deque()  # background task closures (tails, next-pair prep)

        def emit_tail_tasks(h, avh, av_sb, l_sb, rl, rlb, o_sb):
            tasks = []
            for m in range(2):
                t0, t1 = m * TH, (m + 1) * TH

                def evac(m=m, t0=t0, t1=t1):
                    nc.scalar.copy(av_sb[:, t0:t1], avh[m][0:64, :])

                def lcopy(m=m, t0=t0, t1=t1):
                    nc.vector.tensor_copy(l_sb[:, t0:t1], avh[m][64:65, :])

                def recip(t0=t0, t1=t1):
                    nc.vector.reciprocal_approx_fast(
                        out=rl[:, t0:t1], in_=l_sb[:, t0:t1]
                    )

                def bcast(t0=t0, t1=t1):
                    nc.gpsimd.partition_broadcast(rlb[:, t0:t1], rl[:, t0:t1])

                def mult_dma(h=h, t0=t0, t1=t1):
                    nc.gpsimd.tensor_mul(
                        o_sb[:, t0:t1], av_sb[:, t0:t1], rlb[:, t0:t1]
                    )
                    nc.gpsimd.dma_start(
                        out=out[h * 64 : (h + 1) * 64, t0:t1], in_=o_sb[:, t0:t1]
                    )

                tasks += [evac, lcopy, recip, bcast, mult_dma]
            return tasks

        for h in range(H):
            p = h // 2
            o = (h % 2) * 64
            pr = pairs[p]
            q = pr["q2h"][o : o + 64, :]
            k = pr["k2h"][o : o + 64, :]
            vt = pr["vt"][h % 2]

            avh = [
                ps_av.tile([128, TH], f32, tag="av", name="av")
                for m in range(2)
            ]
            av_sb = out_pool.tile([64, T], f32, tag="avsb", name="avsb")
            l_sb = l_pool.tile([1, T], f32, tag="lsb", name="lsb")
            rl = l_pool.tile([1, T], f32, tag="rl", name="rl")
            rlb = l_pool.tile([64, T], f32, tag="rlb", name="rlb")
            o_sb = out_pool.tile([64, T], f32, tag="osb", name="osb")

            if h % 2 == 1 and p + 1 < 4:
                bg.extend(emit_prep_tasks(p + 1, staged=False))

            for j in range(NCH):
                kj = k[:, j * 128 : (j + 1) * 128]
                pts = []
                for m in range(2):
                    sc = ps_sc.tile([128, TH], f32, tag="sc", name="sc")
                    for s0 in range(0, TH, MM_MOV):
                        nc.tensor.matmul(
                            sc[:, s0 : s0 + MM_MOV],
                            kj,
                            q[:, m * TH + s0 : m * TH + s0 + MM_MOV],
                            start=True,
                            stop=True,
                        )
                    pt = pt_pool.tile([128, TH], bf16, name="pt")
                    pts.append(pt)
                    if EXP_ENGINES[m] == "act":
                        nc.scalar.activation(pt, sc, Exp, scale=0.125)
                    else:
                        nc.vector.tensor_scalar(
                            pt.bitcast(i16), sc, SCH_A, SCH_B, mul_op, add_op
                        )

                def av_emit(j=j, pts=pts, avh=avh, vt=vt):
                    for m in range(2):
                        for s0 in range(0, TH, MM_MOV):
                            nc.tensor.matmul(
                                avh[m][0:65, s0 : s0 + MM_MOV],
                                vt[:, j, :],
                                pts[m][:, s0 : s0 + MM_MOV],
                                start=(j == 0),
                                stop=(j == NCH - 1),
                                skip_group_check=True,
                            )

                pending_av.append(av_emit)
                if len(pending_av) > 1:
                    pending_av.popleft()()

                # 1-2 background tasks per chunk
                nbg = 2 if len(bg) > NCH - j else 1
                for _ in range(min(nbg, len(bg))):
                    bg.popleft()()

            bg.extend(emit_tail_tasks(h, avh, av_sb, l_sb, rl, rlb, o_sb))

        # flush: last head's AV_15 + its tail
        while pending_av:
            pending_av.popleft()()
        while bg:
            bg.popleft()()

    nc.finalize()
    return nc


def _get_nc():
    if "nc" not in _CACHE:
        _CACHE["nc"] = _build_nc()
    return _CACHE["nc"]


def _run(qkv_full, trace=False, tmpdir=None):
    """qkv_full: (8, 1536, 2048) f32. Returns (out (8,512,2048) f32, exec_ns)."""
    from concourse.bass_utils import run_bass_kernel_spmd

    nc = _get_nc()
    qkv_full = np.ascontiguousarray(np.asarray(qkv_full, dtype=np.float32))
    in_maps = [{"qkv": qkv_full[i]} for i in range(B)]
    res = run_bass_kernel_spmd(
        nc, in_maps, core_ids=list(range(B)), trace=trace, tmpdir=tmpdir
    )
    outs = np.stack([np.asarray(res.results[i]["out"]) for i in range(B)], axis=0)
    return outs, res.exec_time_ns


def kernel(qkv, n_heads=8):
    out, _ = _run(qkv)
    return out.astype(np.float32)
